# revision 23
# baseline (speedup 1.0000x reference)
"""Trainium2 Bass kernel for nn_MultiHeadDecoder (sparse neighbour compat + MLP + softmax).

Strategy (data-parallel over batch, 8 batches per core):
 - Host: decompose the `rec` permutation into cycles and lay nodes out in tour
   order (with per-cycle pad columns) so predecessor / succ^2 lookups become
   free-dim column shifts on-chip.  The per-core shard is shipped pre-gathered,
   feature-major and in bf16: hemt[b] = h_em[b][order].T  ([128, PEXT+4], last
   4 cols zero so every shifted elementwise op is a single full-width op).
 - Algebra folding (host, float64): the reference's per-head Q/K projections of
   h = h_em @ Wn.T + g-proj collapse into one bilinear form per head:
       compat[pos p] = (A_h[:,p-1]+c_h).F[:,p] + (A_h[:,p]+c_h).E[:,p+2]  (+s)
   where A_h = Mt_h^T E, Mt_h = Wn^T Wq_h Wk_h^T Wn, F = E - shift2(E), and the
   per-batch scalars c_h (from the graph-max projection) ride the PSUM->SBUF
   drain while s folds into the first MLP bias.
 - Device: everything on the PE runs in bf16 (1 cyc/row vs 4 for fp32): one
   128x128 bf16 matmul per head for A, drains split ACT(h<3)/DVE(h=3), bf16 2x
   products on DVE, per-position dot reduction as TensorE column-sum matmuls
   (lhsT=hmask) writing compat feature-major [4 heads, positions], GPSIMD
   ap_gather to join pickup/delivery tour positions into node order, then the
   12->32->32->1 MLP (bf16 matmuls) + tanh + softmax.
"""
import os
import sys
from contextlib import ExitStack

import numpy as np

for _p in ("/opt/trn_rl_repo", "/root/.axon_site/_ro/trn_rl_repo"):
    if os.path.isdir(_p) and _p not in sys.path:
        sys.path.insert(0, _p)

import concourse.bacc as bacc
import concourse.bass as bass
import concourse.mybir as mybir
import concourse.tile as tile
from concourse.bass_utils import run_bass_kernel_spmd
from concourse.library_config import mlp as _mlp_lib

np_bf16 = "float16"  # fp16: same PE/DVE speed class as bf16, 8x mantissa

F32 = mybir.dt.float32
BF16 = mybir.dt.float16
BS, GS, D, NH = 64, 2001, 128, 4
N = GS // 2                 # 1000
NCORES = 8
BPC = BS // NCORES          # 8 batches per core
PEXT = 2048                 # extended tour positions (3 pads/cycle; grown if needed)
EPAD = 4                    # zero guard cols so shifted ops are single full-width
NIDX = 1008                 # padded gather count (>= N, %16 == 0)
IDXW = NIDX // 16           # 63
MLP_CHUNKS = [(0, 512), (512, 488)]

_CACHE = {}


def _chunks():
    out = []
    c0 = 0
    while c0 < PEXT:
        out.append((c0, min(512, PEXT - c0)))
        c0 += 512
    return out


# fp16 const blob layout (free-dim offsets, all slices base partition 0):
# mt 0:512, hmask 512:528, w1p 528:560, w1d 560:592, w1s 592:624,
# w2t 624:656, w3t 656:657
CB16 = 657
# f32 const blob [32, 10]: b1e 0:8, b2 8:9, b3 9:10 (row 0)
CB32 = 10
# per-batch int16 input blob [128, 1136]: sig(fp16) rows 0-3 cols 0:1000,
# pdidx rows 0-15 cols 1000:1126, cvec(f32) rows 0-127 cols 1128:1136
IBW = 1136


def _build_nc():
    CHUNKS = _chunks()
    nc = bacc.Bacc(None, target_bir_lowering=False, debug=False)
    hemt_d = nc.dram_tensor("hemt", [BPC, 128, PEXT + EPAD], BF16, kind="ExternalInput")
    inb_d = nc.dram_tensor("inb", [BPC, 128, IBW], mybir.dt.int16, kind="ExternalInput")
    cb16_d = nc.dram_tensor("cb16", [128, CB16], BF16, kind="ExternalInput")
    cb32_d = nc.dram_tensor("cb32", [32, CB32], F32, kind="ExternalInput")
    out_d = nc.dram_tensor("out", [BPC, N], F32, kind="ExternalOutput")

    with tile.TileContext(nc) as tc:
        with ExitStack() as ctx:
            const = ctx.enter_context(tc.tile_pool(name="const", bufs=1))
            inpool = ctx.enter_context(tc.tile_pool(name="inpool", bufs=3))
            epool = ctx.enter_context(tc.tile_pool(name="epool", bufs=2))
            apool = ctx.enter_context(tc.tile_pool(name="apool", bufs=2))  # per-head A'
            ppool = ctx.enter_context(tc.tile_pool(name="ppool", bufs=2))
            spool = ctx.enter_context(tc.tile_pool(name="spool", bufs=3))
            a_ps = ctx.enter_context(tc.tile_pool(name="a_ps", bufs=2, space="PSUM"))
            cs_ps = ctx.enter_context(tc.tile_pool(name="cs_ps", bufs=2, space="PSUM"))
            mlp_ps = ctx.enter_context(tc.tile_pool(name="mlp_ps", bufs=2, space="PSUM"))

            nc.gpsimd.load_library(_mlp_lib)

            cb16_sb = const.tile([128, CB16], BF16)
            nc.sync.dma_start(out=cb16_sb[:], in_=cb16_d[:])
            cb32_sb = const.tile([32, CB32], F32)
            nc.sync.dma_start(out=cb32_sb[:], in_=cb32_d[:])
            mt_sb = cb16_sb[:, 0:512]
            hmask_sb = cb16_sb[:, 512:528]
            w1p_sb = cb16_sb[0:4, 528:560]
            w1d_sb = cb16_sb[0:4, 560:592]
            w1s_sb = cb16_sb[0:4, 592:624]
            w2t_sb = cb16_sb[0:32, 624:656]
            w3t_sb = cb16_sb[0:32, 656:657]
            b1e_sb = cb32_sb[:, 0:8]
            b2_sb = cb32_sb[:, 8:9]
            b3_sb = cb32_sb[0:1, 9:10]
            compat2 = []
            for i in range(3):
                t = const.tile([16, PEXT], F32, name=f"compat{i}")
                nc.gpsimd.memset(t[:], 0.0)
                compat2.append(t)

            # Software-pipelined emission: batch b's compute phase (DMA ->
            # A/drains -> products -> colsum -> compat -> gather) is emitted
            # before batch b-1's tail phase (pd convert -> MLP -> exp -> out),
            # so each engine's in-order queue never stalls on a dep that a
            # later-emitted instruction would have satisfied sooner.
            tails = {}
            mids = {}

            def compute_phase(b):
                inb_sb = inpool.tile([128, IBW], mybir.dt.int16)
                ef = epool.tile([128, 2 * PEXT + EPAD], BF16)
                e_bf = ef[:, 0:PEXT + EPAD]
                nc.sync.dma_start(out=e_bf, in_=hemt_d[b])
                nc.sync.dma_start(out=inb_sb[:], in_=inb_d[b])
                sig_sb = inb_sb[0:4, 0:N].bitcast(BF16)
                pdidx_sb = inb_sb[0:16, 1000:1000 + 2 * IDXW]
                cv_sb = inb_sb[:, 1128:1136].bitcast(F32)

                # fm1[:, j] = F[:, j+1] = E[:, j+1] - E[:, j+3]  (stored shifted
                # by -1 so every product op is aligned); guard cols are zero.
                # fm1 lives in the same tile as E so one 2-row AP can feed both
                # product rows of a single DVE op.
                fm1 = ef[:, PEXT + EPAD:2 * PEXT + EPAD]
                nc.vector.tensor_sub(fm1[:, 0:PEXT], ef[:, 1:PEXT + 1],
                                     ef[:, 3:PEXT + 3])

                # A'_h = Mt_h^T E + c_h, drained PSUM->SBUF as fp16 (ACT h<3,
                # DVE h=3), immediately consumed by the fused product op.
                p_sb = ppool.tile([128, 2 * NH, PEXT], BF16)
                for h in range(NH):
                    cv = cv_sb[:, h:h + 1]
                    a_bf = apool.tile([128, PEXT], BF16, tag="ah")
                    for pair0 in range(0, PEXT, 1024):
                        pw = min(1024, PEXT - pair0)
                        ap = a_ps.tile([128, 1024], F32, space="PSUM", tag="a")
                        for (c0, w) in [(c, min(512, pw - (c - pair0)))
                                        for c in range(pair0, pair0 + pw, 512)]:
                            nc.tensor.matmul(out=ap[:, c0 - pair0:c0 - pair0 + w],
                                             lhsT=mt_sb[:, h * 128:(h + 1) * 128],
                                             rhs=e_bf[:, c0:c0 + w], start=True, stop=True)
                        if h < 3:
                            nc.scalar.add(out=a_bf[:, pair0:pair0 + pw],
                                          in_=ap[:, :pw], add=cv)
                        else:
                            nc.vector.tensor_scalar_add(a_bf[:, pair0:pair0 + pw],
                                                        ap[:, :pw], cv)
                    # fused products (fp16, 2x), 1024-chunks so the column-sum
                    # matmuls pipeline into them:
                    # p_sb[:, 2h, j]   = A'_h[:, j] * F[:, j+1]
                    # p_sb[:, 2h+1, j] = A'_h[:, j] * E[:, j+2]
                    APc = type(fm1)
                    for pc in range(0, PEXT, 1024):
                        pw = min(1024, PEXT - pc)
                        ein = ef[:, pc + 2:pc + 2 + pw]
                        in1 = APc(ein.tensor, ein.offset,
                                  [ein.ap[0], [PEXT + EPAD - 2, 2], [1, pw]])
                        pout = p_sb[:, 2 * h + 1, pc:pc + pw]
                        out2 = APc(pout.tensor, pout.offset,
                                   [pout.ap[0], [-PEXT, 2], [1, pw]])
                        ain = a_bf[:, pc:pc + pw]
                        in0 = APc(ain.tensor, ain.offset,
                                  [ain.ap[0], [0, 2], [1, pw]])
                        nc.vector.tensor_mul(out2, in0, in1)

                mids[b] = (p_sb, pdidx_sb, sig_sb)

            def colsum_phase(b):
                p_sb, pdidx_sb, sig_sb = mids.pop(b)
                # compat[h, pos p] = colsum(P2_h)[p] + colsum(P1_h)[p-1]
                compat_sb = compat2[b % 3]
                for (c0, w) in CHUNKS:
                    cs = cs_ps.tile([4, 512], F32, space="PSUM", tag="cs")
                    for h in range(NH):
                        mk = hmask_sb[:, h * NH:(h + 1) * NH]
                        if h < NH - 1:
                            nc.tensor.matmul(out=cs[:, :w], lhsT=mk,
                                             rhs=p_sb[:, 2 * h + 1, c0:c0 + w],
                                             start=(h == 0), stop=False)
                            t1 = (cs[:, 1:w], p_sb[:, 2 * h, 0:w - 1]) if c0 == 0 else \
                                 (cs[:, :w], p_sb[:, 2 * h, c0 - 1:c0 - 1 + w])
                            nc.tensor.matmul(out=t1[0], lhsT=mk, rhs=t1[1],
                                             start=False, stop=False,
                                             skip_group_check=True)
                        else:
                            t1 = (cs[:, 1:w], p_sb[:, 2 * h, 0:w - 1]) if c0 == 0 else \
                                 (cs[:, :w], p_sb[:, 2 * h, c0 - 1:c0 - 1 + w])
                            nc.tensor.matmul(out=t1[0], lhsT=mk, rhs=t1[1],
                                             start=False, stop=False,
                                             skip_group_check=True)
                            nc.tensor.matmul(out=cs[:, :w], lhsT=mk,
                                             rhs=p_sb[:, 2 * h + 1, c0:c0 + w],
                                             start=False, stop=True)
                    nc.scalar.copy(out=compat_sb[0:4, c0:c0 + w], in_=cs[:, :w])

                pd_g = spool.tile([16, 2 * NIDX], F32)
                nc.gpsimd.ap_gather(pd_g[:], compat_sb[:], pdidx_sb[:],
                                    channels=16, num_elems=PEXT, d=1, num_idxs=2 * NIDX)
                pd_bf = spool.tile([4, 2 * NIDX], BF16)
                nc.gpsimd.tensor_copy(out=pd_bf[:], in_=pd_g[0:4, :])
                tails[b] = (pd_bf, sig_sb)


            def tail_phase(b):
                pd_bf, sig_sb = tails.pop(b)

                x1_sb = spool.tile([32, N], BF16)
                x2_sb = spool.tile([32, N], BF16)
                tab_sb = spool.tile([1, N], F32)
                for (c0, w) in MLP_CHUNKS:
                    x1p = mlp_ps.tile([32, 512], F32, space="PSUM", tag="m")
                    nc.tensor.matmul(out=x1p[:, :w], lhsT=w1p_sb, rhs=pd_bf[:, c0:c0 + w],
                                     start=True, stop=False)
                    nc.tensor.matmul(out=x1p[:, :w], lhsT=w1d_sb,
                                     rhs=pd_bf[:, NIDX + c0:NIDX + c0 + w],
                                     start=False, stop=False)
                    nc.tensor.matmul(out=x1p[:, :w], lhsT=w1s_sb, rhs=sig_sb[:, c0:c0 + w],
                                     start=False, stop=True)
                    nc.scalar.activation(out=x1_sb[:, c0:c0 + w], in_=x1p[:, :w],
                                         func=mybir.ActivationFunctionType.Relu,
                                         bias=b1e_sb[:, b:b + 1], scale=1.0)
                for (c0, w) in MLP_CHUNKS:
                    x2p = mlp_ps.tile([32, 512], F32, space="PSUM", tag="m")
                    nc.tensor.matmul(out=x2p[:, :w], lhsT=w2t_sb, rhs=x1_sb[:, c0:c0 + w],
                                     start=True, stop=True)
                    nc.scalar.activation(out=x2_sb[:, c0:c0 + w], in_=x2p[:, :w],
                                         func=mybir.ActivationFunctionType.Relu,
                                         bias=b2_sb, scale=1.0)
                for (c0, w) in MLP_CHUNKS:
                    tp3 = mlp_ps.tile([1, 512], F32, space="PSUM", tag="m")
                    nc.tensor.matmul(out=tp3[:, :w], lhsT=w3t_sb, rhs=x2_sb[:, c0:c0 + w],
                                     start=True, stop=True)
                    nc.scalar.activation(out=tab_sb[:, c0:c0 + w], in_=tp3[:, :w],
                                         func=mybir.ActivationFunctionType.Tanh,
                                         bias=b3_sb, scale=1.0)

                # exp(6*tanh); normalization happens on the host (sum + divide)
                ex_sb = spool.tile([1, N], F32)
                nc.scalar.activation(out=ex_sb[:], in_=tab_sb[:],
                                     func=mybir.ActivationFunctionType.Exp,
                                     bias=0.0, scale=6.0)
                nc.sync.dma_start(out=out_d[b:b + 1, :], in_=ex_sb[:])

            for b in range(BPC + 2):
                if b < BPC:
                    compute_phase(b)
                if 1 <= b <= BPC:
                    colsum_phase(b - 1)
                if b >= 2:
                    tail_phase(b - 2)
    nc.compile()
    return nc


def _decompose(perm):
    visited = np.zeros(GS, bool)
    order = []
    real = []
    for start in range(GS):
        if visited[start]:
            continue
        cyc = [start]
        visited[start] = True
        nxt = int(perm[start])
        while nxt != start:
            cyc.append(nxt)
            visited[nxt] = True
            nxt = int(perm[nxt])
        L = len(cyc)
        order.extend([cyc[-1]] + cyc + [cyc[0 % L], cyc[1 % L]])
        real.extend([False] + [True] * L + [False] * 2)
    assert len(order) <= PEXT, f"too many cycles: ext len {len(order)}"
    pad = PEXT - len(order)
    order.extend([0] * pad)
    real.extend([False] * pad)
    return np.asarray(order, np.int64), np.asarray(real, bool)


def _ext_len(perm):
    visited = np.zeros(GS, bool)
    ncyc = 0
    for start in range(GS):
        if not visited[start]:
            ncyc += 1
            visited[start] = True
            nxt = int(perm[start])
            while nxt != start:
                visited[nxt] = True
                nxt = int(perm[nxt])
    return GS + 3 * ncyc


def _idx_tile(ppos, dpos):
    idx = np.zeros(2 * NIDX, np.int16)
    idx[:ppos.shape[0]] = ppos
    idx[NIDX:NIDX + dpos.shape[0]] = dpos
    return idx.reshape(2 * IDXW, 16).T.copy()


def _host_prep(inputs):
    h_em = np.asarray(inputs["h_em"], np.float32)
    rec = np.asarray(inputs["rec"], np.int64)
    sig = np.ascontiguousarray(np.asarray(inputs["selection_sig"], np.float32))
    Wn = np.asarray(inputs["W_node"], np.float64)
    Wg = np.asarray(inputs["W_graph"], np.float64)
    WQ = np.asarray(inputs["W_Q"], np.float64)
    WK = np.asarray(inputs["W_K"], np.float64)
    w1 = np.asarray(inputs["agg_w1"], np.float64)
    b1 = np.asarray(inputs["agg_b1"], np.float64)
    w2 = np.asarray(inputs["agg_w2"], np.float32)
    b2 = np.asarray(inputs["agg_b2"], np.float32)
    w3 = np.asarray(inputs["agg_w3"], np.float32)
    b3 = np.asarray(inputs["agg_b3"], np.float32)

    Mt = np.zeros((NH, D, D), np.float64)
    C = np.zeros((NH, D, D), np.float64)
    S = np.zeros((NH, D, D), np.float64)
    for h in range(NH):
        M = WQ[h] @ WK[h].T
        Mt[h] = Wn.T @ M @ Wn
        C[h] = Wn.T @ (M + M.T) @ Wg
        S[h] = Wg.T @ M @ Wg
    mt = np.concatenate([Mt[h].astype(np.float32) for h in range(NH)],
                        axis=1).astype(np.float16)

    g = h_em.max(axis=1).astype(np.float64)                      # (BS, D)
    cvec = np.einsum("hdf,bf->bdh", C, g).astype(np.float32)     # (BS, D, NH)
    svec = np.einsum("bd,hdf,bf->bh", g, S, g)                   # (BS, NH)
    b1_eff = (b1[None, :] + svec @ (w1[:, 0:4] + w1[:, 4:8]).T).astype(np.float32)

    w1f = w1.astype(np.float32)
    hmask = np.zeros((128, NH * NH), np.float32)
    for h in range(NH):
        hmask[:, h * NH + h] = 1.0

    cb16 = np.zeros((128, CB16), np.float16)
    cb16[:, 0:512] = mt
    cb16[:, 512:528] = hmask.astype(np.float16)
    cb16[0:4, 528:560] = w1f[:, 0:4].T.astype(np.float16)
    cb16[0:4, 560:592] = w1f[:, 4:8].T.astype(np.float16)
    cb16[0:4, 592:624] = w1f[:, 8:12].T.astype(np.float16)
    cb16[0:32, 624:656] = w2.T.astype(np.float16)
    cb16[0:32, 656] = w3.reshape(32).astype(np.float16)

    sig16 = sig.astype(np.float16)

    in_maps = []
    for core in range(NCORES):
        b0 = core * BPC
        hemt = np.zeros((BPC, 128, PEXT + EPAD), np.float16)
        inb = np.zeros((BPC, 128, IBW), np.int16)
        for bl in range(BPC):
            order, real = _decompose(rec[b0 + bl])
            hemt[bl, :, :PEXT] = h_em[b0 + bl][order].T.astype(np.float16)
            pon = np.zeros(GS, np.int64)
            pon[order[real]] = np.nonzero(real)[0]
            inb[bl, 0:4, 0:N] = np.ascontiguousarray(sig16[b0 + bl]).view(np.int16)
            inb[bl, 0:16, 1000:1000 + 2 * IDXW] = _idx_tile(
                pon[1:N + 1], pon[N + 1:2 * N + 1])
            inb[bl, :, 1128:1136] = np.ascontiguousarray(cvec[b0 + bl]).view(np.int16)
        cb32 = np.zeros((32, CB32), np.float32)
        cb32[:, 0:8] = b1_eff[b0:b0 + BPC].T
        cb32[:, 8] = b2
        cb32[0, 9] = b3[0]
        in_maps.append({"hemt": hemt, "inb": inb, "cb16": cb16, "cb32": cb32})
    return in_maps


def kernel(**inputs) -> np.ndarray:
    global PEXT
    rec = np.asarray(inputs["rec"], np.int64)
    need = max(_ext_len(rec[b]) for b in range(rec.shape[0]))
    want = max(2048, -(-need // 512) * 512)
    if want != PEXT or "nc" not in _CACHE:
        PEXT = want
        _CACHE["nc"] = _build_nc()
    nc = _CACHE["nc"]
    in_maps = _host_prep(inputs)
    res = run_bass_kernel_spmd(nc, in_maps, list(range(NCORES)))
    ex = np.concatenate([res.results[i]["out"] for i in range(NCORES)], axis=0)
    return (ex / ex.sum(axis=1, keepdims=True)).astype(np.float32)


# revision 24
# speedup vs baseline: 1.0686x; 1.0686x over previous
"""Trainium2 Bass kernel for nn_MultiHeadDecoder (sparse neighbour compat + MLP + softmax).

Strategy (data-parallel over batch, 8 batches per core):
 - Host: decompose the `rec` permutation into cycles and lay nodes out in tour
   order (with per-cycle pad columns) so predecessor / succ^2 lookups become
   free-dim column shifts on-chip.  The per-core shard is shipped pre-gathered,
   feature-major and in bf16: hemt[b] = h_em[b][order].T  ([128, PEXT+4], last
   4 cols zero so every shifted elementwise op is a single full-width op).
 - Algebra folding (host, float64): the reference's per-head Q/K projections of
   h = h_em @ Wn.T + g-proj collapse into one bilinear form per head:
       compat[pos p] = (A_h[:,p-1]+c_h).F[:,p] + (A_h[:,p]+c_h).E[:,p+2]  (+s)
   where A_h = Mt_h^T E, Mt_h = Wn^T Wq_h Wk_h^T Wn, F = E - shift2(E), and the
   per-batch scalars c_h (from the graph-max projection) ride the PSUM->SBUF
   drain while s folds into the first MLP bias.
 - Device: everything on the PE runs in bf16 (1 cyc/row vs 4 for fp32): one
   128x128 bf16 matmul per head for A, drains split ACT(h<3)/DVE(h=3), bf16 2x
   products on DVE, per-position dot reduction as TensorE column-sum matmuls
   (lhsT=hmask) writing compat feature-major [4 heads, positions], GPSIMD
   ap_gather to join pickup/delivery tour positions into node order, then the
   12->32->32->1 MLP (bf16 matmuls) + tanh + softmax.
"""
import os
import sys
from contextlib import ExitStack

import numpy as np

for _p in ("/opt/trn_rl_repo", "/root/.axon_site/_ro/trn_rl_repo"):
    if os.path.isdir(_p) and _p not in sys.path:
        sys.path.insert(0, _p)

import concourse.bacc as bacc
import concourse.bass as bass
import concourse.mybir as mybir
import concourse.tile as tile
from concourse.bass_utils import run_bass_kernel_spmd
from concourse.library_config import mlp as _mlp_lib

np_bf16 = "float16"  # fp16: same PE/DVE speed class as bf16, 8x mantissa

F32 = mybir.dt.float32
BF16 = mybir.dt.float16
BS, GS, D, NH = 64, 2001, 128, 4
N = GS // 2                 # 1000
NCORES = 8
BPC = BS // NCORES          # 8 batches per core
PEXT = 2048                 # extended tour positions (3 pads/cycle; grown if needed)
EPAD = 4                    # zero guard cols so shifted ops are single full-width
NIDX = 1008                 # padded gather count (>= N, %16 == 0)
IDXW = NIDX // 16           # 63
MLP_CHUNKS = [(0, 512), (512, 488)]

_CACHE = {}


def _chunks():
    out = []
    c0 = 0
    while c0 < PEXT:
        out.append((c0, min(512, PEXT - c0)))
        c0 += 512
    return out


# fp16 const blob layout (free-dim offsets, all slices base partition 0):
# mt 0:512, hmask 512:528, w1p 528:560, w1d 560:592, w1s 592:624,
# w2t 624:656, w3t 656:657
CB16 = 657
# f32 const blob [32, 10]: b1e 0:8, b2 8:9, b3 9:10 (row 0)
CB32 = 10
# per-batch int16 input blob [128, 1136]: sig(fp16) rows 0-3 cols 0:1000,
# pdidx rows 0-15 cols 1000:1126, cvec(f32) rows 0-127 cols 1128:1136
IBW = 1136


def _build_nc():
    CHUNKS = _chunks()
    nc = bacc.Bacc(None, target_bir_lowering=False, debug=False)
    hemt_d = nc.dram_tensor("hemt", [BPC, 128, PEXT + EPAD], BF16, kind="ExternalInput")
    inb_d = nc.dram_tensor("inb", [BPC, 128, IBW], mybir.dt.int16, kind="ExternalInput")
    cb16_d = nc.dram_tensor("cb16", [128, CB16], BF16, kind="ExternalInput")
    cb32_d = nc.dram_tensor("cb32", [32, CB32], F32, kind="ExternalInput")
    out_d = nc.dram_tensor("out", [BPC, N], F32, kind="ExternalOutput")

    with tile.TileContext(nc) as tc:
        with ExitStack() as ctx:
            const = ctx.enter_context(tc.tile_pool(name="const", bufs=1))
            inpool = ctx.enter_context(tc.tile_pool(name="inpool", bufs=3))
            epool = ctx.enter_context(tc.tile_pool(name="epool", bufs=2))
            apool = ctx.enter_context(tc.tile_pool(name="apool", bufs=2))  # per-head A'
            ppool = ctx.enter_context(tc.tile_pool(name="ppool", bufs=2))
            spool = ctx.enter_context(tc.tile_pool(name="spool", bufs=3))
            a_ps = ctx.enter_context(tc.tile_pool(name="a_ps", bufs=2, space="PSUM"))
            cs_ps = ctx.enter_context(tc.tile_pool(name="cs_ps", bufs=2, space="PSUM"))
            mlp_ps = ctx.enter_context(tc.tile_pool(name="mlp_ps", bufs=2, space="PSUM"))

            nc.gpsimd.load_library(_mlp_lib)

            cb16_sb = const.tile([128, CB16], BF16)
            nc.sync.dma_start(out=cb16_sb[:], in_=cb16_d[:])
            cb32_sb = const.tile([32, CB32], F32)
            nc.sync.dma_start(out=cb32_sb[:], in_=cb32_d[:])
            mt_sb = cb16_sb[:, 0:512]
            hmask_sb = cb16_sb[:, 512:528]
            w1p_sb = cb16_sb[0:4, 528:560]
            w1d_sb = cb16_sb[0:4, 560:592]
            w1s_sb = cb16_sb[0:4, 592:624]
            w2t_sb = cb16_sb[0:32, 624:656]
            w3t_sb = cb16_sb[0:32, 656:657]
            b1e_sb = cb32_sb[:, 0:8]
            b2_sb = cb32_sb[:, 8:9]
            b3_sb = cb32_sb[0:1, 9:10]
            warm = const.tile([1, 1], F32)
            for fn in (mybir.ActivationFunctionType.Relu,
                       mybir.ActivationFunctionType.Tanh,
                       mybir.ActivationFunctionType.Exp):
                nc.scalar.activation(out=warm[:], in_=warm[:], func=fn,
                                     bias=0.0, scale=1.0)
            compat2 = []
            for i in range(3):
                t = const.tile([16, PEXT], F32, name=f"compat{i}")
                nc.gpsimd.memset(t[:], 0.0)
                compat2.append(t)

            # Software-pipelined emission: batch b's compute phase (DMA ->
            # A/drains -> products -> colsum -> compat -> gather) is emitted
            # before batch b-1's tail phase (pd convert -> MLP -> exp -> out),
            # so each engine's in-order queue never stalls on a dep that a
            # later-emitted instruction would have satisfied sooner.
            tails = {}
            mids = {}

            def compute_phase(b):
                inb_sb = inpool.tile([128, IBW], mybir.dt.int16)
                ef = epool.tile([128, 2 * PEXT + EPAD], BF16)
                e_bf = ef[:, 0:PEXT + EPAD]
                nc.sync.dma_start(out=e_bf, in_=hemt_d[b])
                nc.sync.dma_start(out=inb_sb[:], in_=inb_d[b])
                sig_sb = inb_sb[0:4, 0:N].bitcast(BF16)
                pdidx_sb = inb_sb[0:16, 1000:1000 + 2 * IDXW]
                cv_sb = inb_sb[:, 1128:1136].bitcast(F32)

                # fm1[:, j] = F[:, j+1] = E[:, j+1] - E[:, j+3]  (stored shifted
                # by -1 so every product op is aligned); guard cols are zero.
                # fm1 lives in the same tile as E so one 2-row AP can feed both
                # product rows of a single DVE op.
                fm1 = ef[:, PEXT + EPAD:2 * PEXT + EPAD]
                nc.vector.tensor_sub(fm1[:, 0:PEXT], ef[:, 1:PEXT + 1],
                                     ef[:, 3:PEXT + 3])

                # A'_h = Mt_h^T E + c_h, drained PSUM->SBUF as fp16 (ACT h<3,
                # DVE h=3), immediately consumed by the fused product op.
                p_sb = ppool.tile([128, 2 * NH, PEXT], BF16)
                for h in range(NH):
                    cv = cv_sb[:, h:h + 1]
                    a_bf = apool.tile([128, PEXT], BF16, tag="ah")
                    for pair0 in range(0, PEXT, 1024):
                        pw = min(1024, PEXT - pair0)
                        ap = a_ps.tile([128, 1024], F32, space="PSUM", tag="a")
                        for (c0, w) in [(c, min(512, pw - (c - pair0)))
                                        for c in range(pair0, pair0 + pw, 512)]:
                            nc.tensor.matmul(out=ap[:, c0 - pair0:c0 - pair0 + w],
                                             lhsT=mt_sb[:, h * 128:(h + 1) * 128],
                                             rhs=e_bf[:, c0:c0 + w], start=True, stop=True)
                        if h < 3:
                            nc.scalar.add(out=a_bf[:, pair0:pair0 + pw],
                                          in_=ap[:, :pw], add=cv)
                        else:
                            nc.vector.tensor_scalar_add(a_bf[:, pair0:pair0 + pw],
                                                        ap[:, :pw], cv)
                    # fused products (fp16, 2x), 1024-chunks so the column-sum
                    # matmuls pipeline into them:
                    # p_sb[:, 2h, j]   = A'_h[:, j] * F[:, j+1]
                    # p_sb[:, 2h+1, j] = A'_h[:, j] * E[:, j+2]
                    APc = type(fm1)
                    for pc in range(0, PEXT, 1024):
                        pw = min(1024, PEXT - pc)
                        ein = ef[:, pc + 2:pc + 2 + pw]
                        in1 = APc(ein.tensor, ein.offset,
                                  [ein.ap[0], [PEXT + EPAD - 2, 2], [1, pw]])
                        pout = p_sb[:, 2 * h + 1, pc:pc + pw]
                        out2 = APc(pout.tensor, pout.offset,
                                   [pout.ap[0], [-PEXT, 2], [1, pw]])
                        ain = a_bf[:, pc:pc + pw]
                        in0 = APc(ain.tensor, ain.offset,
                                  [ain.ap[0], [0, 2], [1, pw]])
                        nc.vector.tensor_mul(out2, in0, in1)

                mids[b] = (p_sb, pdidx_sb, sig_sb)

            def colsum_phase(b):
                p_sb, pdidx_sb, sig_sb = mids.pop(b)
                # compat[h, pos p] = colsum(P2_h)[p] + colsum(P1_h)[p-1]
                compat_sb = compat2[b % 3]
                for (c0, w) in CHUNKS:
                    cs = cs_ps.tile([4, 512], F32, space="PSUM", tag="cs")
                    for h in range(NH):
                        mk = hmask_sb[:, h * NH:(h + 1) * NH]
                        if h < NH - 1:
                            nc.tensor.matmul(out=cs[:, :w], lhsT=mk,
                                             rhs=p_sb[:, 2 * h + 1, c0:c0 + w],
                                             start=(h == 0), stop=False)
                            t1 = (cs[:, 1:w], p_sb[:, 2 * h, 0:w - 1]) if c0 == 0 else \
                                 (cs[:, :w], p_sb[:, 2 * h, c0 - 1:c0 - 1 + w])
                            nc.tensor.matmul(out=t1[0], lhsT=mk, rhs=t1[1],
                                             start=False, stop=False,
                                             skip_group_check=True)
                        else:
                            t1 = (cs[:, 1:w], p_sb[:, 2 * h, 0:w - 1]) if c0 == 0 else \
                                 (cs[:, :w], p_sb[:, 2 * h, c0 - 1:c0 - 1 + w])
                            nc.tensor.matmul(out=t1[0], lhsT=mk, rhs=t1[1],
                                             start=False, stop=False,
                                             skip_group_check=True)
                            nc.tensor.matmul(out=cs[:, :w], lhsT=mk,
                                             rhs=p_sb[:, 2 * h + 1, c0:c0 + w],
                                             start=False, stop=True)
                    nc.scalar.copy(out=compat_sb[0:4, c0:c0 + w], in_=cs[:, :w])

                pd_g = spool.tile([16, 2 * NIDX], F32)
                nc.gpsimd.ap_gather(pd_g[:], compat_sb[:], pdidx_sb[:],
                                    channels=16, num_elems=PEXT, d=1, num_idxs=2 * NIDX)
                tails[b] = (pd_g, sig_sb)


            def tail_phase(b):
                pd_g, sig_sb = tails.pop(b)
                pd_bf = spool.tile([4, 2 * NIDX], BF16)
                nc.vector.tensor_copy(out=pd_bf[:], in_=pd_g[0:4, :])

                x1_sb = spool.tile([32, N], BF16)
                x2_sb = spool.tile([32, N], BF16)
                tab_sb = spool.tile([1, N], F32)
                for (c0, w) in MLP_CHUNKS:
                    x1p = mlp_ps.tile([32, 512], F32, space="PSUM", tag="m")
                    nc.tensor.matmul(out=x1p[:, :w], lhsT=w1p_sb, rhs=pd_bf[:, c0:c0 + w],
                                     start=True, stop=False)
                    nc.tensor.matmul(out=x1p[:, :w], lhsT=w1d_sb,
                                     rhs=pd_bf[:, NIDX + c0:NIDX + c0 + w],
                                     start=False, stop=False)
                    nc.tensor.matmul(out=x1p[:, :w], lhsT=w1s_sb, rhs=sig_sb[:, c0:c0 + w],
                                     start=False, stop=True)
                    nc.scalar.activation(out=x1_sb[:, c0:c0 + w], in_=x1p[:, :w],
                                         func=mybir.ActivationFunctionType.Relu,
                                         bias=b1e_sb[:, b:b + 1], scale=1.0)
                for (c0, w) in MLP_CHUNKS:
                    x2p = mlp_ps.tile([32, 512], F32, space="PSUM", tag="m")
                    nc.tensor.matmul(out=x2p[:, :w], lhsT=w2t_sb, rhs=x1_sb[:, c0:c0 + w],
                                     start=True, stop=True)
                    nc.scalar.activation(out=x2_sb[:, c0:c0 + w], in_=x2p[:, :w],
                                         func=mybir.ActivationFunctionType.Relu,
                                         bias=b2_sb, scale=1.0)
                for (c0, w) in MLP_CHUNKS:
                    tp3 = mlp_ps.tile([1, 512], F32, space="PSUM", tag="m")
                    nc.tensor.matmul(out=tp3[:, :w], lhsT=w3t_sb, rhs=x2_sb[:, c0:c0 + w],
                                     start=True, stop=True)
                    nc.scalar.activation(out=tab_sb[:, c0:c0 + w], in_=tp3[:, :w],
                                         func=mybir.ActivationFunctionType.Tanh,
                                         bias=b3_sb, scale=1.0)

                # exp(6*tanh); normalization happens on the host (sum + divide)
                ex_sb = spool.tile([1, N], F32)
                nc.scalar.activation(out=ex_sb[:], in_=tab_sb[:],
                                     func=mybir.ActivationFunctionType.Exp,
                                     bias=0.0, scale=6.0)
                nc.sync.dma_start(out=out_d[b:b + 1, :], in_=ex_sb[:])

            for b in range(BPC + 2):
                if b < BPC:
                    compute_phase(b)
                if 1 <= b <= BPC:
                    colsum_phase(b - 1)
                if b >= 2:
                    tail_phase(b - 2)
    nc.compile()
    return nc


def _decompose(perm):
    visited = np.zeros(GS, bool)
    order = []
    real = []
    for start in range(GS):
        if visited[start]:
            continue
        cyc = [start]
        visited[start] = True
        nxt = int(perm[start])
        while nxt != start:
            cyc.append(nxt)
            visited[nxt] = True
            nxt = int(perm[nxt])
        L = len(cyc)
        order.extend([cyc[-1]] + cyc + [cyc[0 % L], cyc[1 % L]])
        real.extend([False] + [True] * L + [False] * 2)
    assert len(order) <= PEXT, f"too many cycles: ext len {len(order)}"
    pad = PEXT - len(order)
    order.extend([0] * pad)
    real.extend([False] * pad)
    return np.asarray(order, np.int64), np.asarray(real, bool)


def _ext_len(perm):
    visited = np.zeros(GS, bool)
    ncyc = 0
    for start in range(GS):
        if not visited[start]:
            ncyc += 1
            visited[start] = True
            nxt = int(perm[start])
            while nxt != start:
                visited[nxt] = True
                nxt = int(perm[nxt])
    return GS + 3 * ncyc


def _idx_tile(ppos, dpos):
    idx = np.zeros(2 * NIDX, np.int16)
    idx[:ppos.shape[0]] = ppos
    idx[NIDX:NIDX + dpos.shape[0]] = dpos
    return idx.reshape(2 * IDXW, 16).T.copy()


def _host_prep(inputs):
    h_em = np.asarray(inputs["h_em"], np.float32)
    rec = np.asarray(inputs["rec"], np.int64)
    sig = np.ascontiguousarray(np.asarray(inputs["selection_sig"], np.float32))
    Wn = np.asarray(inputs["W_node"], np.float64)
    Wg = np.asarray(inputs["W_graph"], np.float64)
    WQ = np.asarray(inputs["W_Q"], np.float64)
    WK = np.asarray(inputs["W_K"], np.float64)
    w1 = np.asarray(inputs["agg_w1"], np.float64)
    b1 = np.asarray(inputs["agg_b1"], np.float64)
    w2 = np.asarray(inputs["agg_w2"], np.float32)
    b2 = np.asarray(inputs["agg_b2"], np.float32)
    w3 = np.asarray(inputs["agg_w3"], np.float32)
    b3 = np.asarray(inputs["agg_b3"], np.float32)

    Mt = np.zeros((NH, D, D), np.float64)
    C = np.zeros((NH, D, D), np.float64)
    S = np.zeros((NH, D, D), np.float64)
    for h in range(NH):
        M = WQ[h] @ WK[h].T
        Mt[h] = Wn.T @ M @ Wn
        C[h] = Wn.T @ (M + M.T) @ Wg
        S[h] = Wg.T @ M @ Wg
    mt = np.concatenate([Mt[h].astype(np.float32) for h in range(NH)],
                        axis=1).astype(np.float16)

    g = h_em.max(axis=1).astype(np.float64)                      # (BS, D)
    cvec = np.einsum("hdf,bf->bdh", C, g).astype(np.float32)     # (BS, D, NH)
    svec = np.einsum("bd,hdf,bf->bh", g, S, g)                   # (BS, NH)
    b1_eff = (b1[None, :] + svec @ (w1[:, 0:4] + w1[:, 4:8]).T).astype(np.float32)

    w1f = w1.astype(np.float32)
    hmask = np.zeros((128, NH * NH), np.float32)
    for h in range(NH):
        hmask[:, h * NH + h] = 1.0

    cb16 = np.zeros((128, CB16), np.float16)
    cb16[:, 0:512] = mt
    cb16[:, 512:528] = hmask.astype(np.float16)
    cb16[0:4, 528:560] = w1f[:, 0:4].T.astype(np.float16)
    cb16[0:4, 560:592] = w1f[:, 4:8].T.astype(np.float16)
    cb16[0:4, 592:624] = w1f[:, 8:12].T.astype(np.float16)
    cb16[0:32, 624:656] = w2.T.astype(np.float16)
    cb16[0:32, 656] = w3.reshape(32).astype(np.float16)

    sig16 = sig.astype(np.float16)

    in_maps = []
    for core in range(NCORES):
        b0 = core * BPC
        hemt = np.zeros((BPC, 128, PEXT + EPAD), np.float16)
        inb = np.zeros((BPC, 128, IBW), np.int16)
        for bl in range(BPC):
            order, real = _decompose(rec[b0 + bl])
            hemt[bl, :, :PEXT] = h_em[b0 + bl][order].T.astype(np.float16)
            pon = np.zeros(GS, np.int64)
            pon[order[real]] = np.nonzero(real)[0]
            inb[bl, 0:4, 0:N] = np.ascontiguousarray(sig16[b0 + bl]).view(np.int16)
            inb[bl, 0:16, 1000:1000 + 2 * IDXW] = _idx_tile(
                pon[1:N + 1], pon[N + 1:2 * N + 1])
            inb[bl, :, 1128:1136] = np.ascontiguousarray(cvec[b0 + bl]).view(np.int16)
        cb32 = np.zeros((32, CB32), np.float32)
        cb32[:, 0:8] = b1_eff[b0:b0 + BPC].T
        cb32[:, 8] = b2
        cb32[0, 9] = b3[0]
        in_maps.append({"hemt": hemt, "inb": inb, "cb16": cb16, "cb32": cb32})
    return in_maps


def kernel(**inputs) -> np.ndarray:
    global PEXT
    rec = np.asarray(inputs["rec"], np.int64)
    need = max(_ext_len(rec[b]) for b in range(rec.shape[0]))
    want = max(2048, -(-need // 512) * 512)
    if want != PEXT or "nc" not in _CACHE:
        PEXT = want
        _CACHE["nc"] = _build_nc()
    nc = _CACHE["nc"]
    in_maps = _host_prep(inputs)
    res = run_bass_kernel_spmd(nc, in_maps, list(range(NCORES)))
    ex = np.concatenate([res.results[i]["out"] for i in range(NCORES)], axis=0)
    return (ex / ex.sum(axis=1, keepdims=True)).astype(np.float32)
